# revision 1
# baseline (speedup 1.0000x reference)
"""Causal self-attention (shared-V, head-weighted sum) on 8 trn2 NeuronCores.

Reference computation (B=2, T=2048, C=1024, H=16, hs=64):
    qk = x @ W_attn + b_attn ; q, k = split(qk)
    att = softmax(causal(q @ k^T / sqrt(hs)))
    y   = sum_h head_weights[h] * (att_h @ x)

Sharding: tensor-parallel over heads. Core i computes heads {2i, 2i+1} for
both batches and returns its partial y; the host sums the 8 partials.

Per-core pipeline (bf16 matmuls, f32 accumulation):
  1. proj: qT/kT [128(=2 heads x 64), B*T] = W_tile^T @ x^T, bias via ACT.
  2. QK:   attT[s,q] psum tiles (K=64 matmuls), exp on ACT (scale=1/8) into
           causally-packed SBUF storage; diagonal blocks masked via DVE mul.
  3. AV:   y[q,c] psum = expT^T @ x_aug where x_aug has a ones column, so the
           softmax denominator comes out of the same matmuls.
  4. norm+combine: one fused DVE op (psum * 1/denom) * w_head per chunk,
           second head added on the Pool engine, DMA to DRAM.
"""

import numpy as np
import ml_dtypes

import concourse.bass as bass
import concourse.bacc as bacc
import concourse.mybir as mybir
import concourse.tile as tile
from concourse.bass_utils import run_bass_kernel_spmd

BF16 = ml_dtypes.bfloat16
F32 = mybir.dt.float32
BF = mybir.dt.bfloat16

B, T, C, H = 2, 2048, 1024, 16
NCORES = 8
HPC = H // NCORES          # heads per core = 2
HS = C // H                # head size = 64
NT = T // 128              # 16 s/q tiles per batch
CAUG = C + 2               # x columns + ones column + pad = 1026
CCH = CAUG // 3            # AV moving-dim chunk = 342
TCH = 512                  # proj/QK moving-dim chunk
NTC = B * T // TCH         # 8 proj t-chunks
NCT = C // 128             # 8 contraction tiles

# causally-packed expT storage: tile j holds q in [128j, 2048) -> offset table
OFF = [0] * NT
for _j in range(1, NT):
    OFF[_j] = OFF[_j - 1] + (T - 128 * (_j - 1))
EXP_COLS = OFF[NT - 1] + (T - 128 * (NT - 1))  # 17408


def _emit(nc, xTt_d, xaug_d, wqk_d, bqk_d, wh_d, mask_d, y_d, tc):
    Ident = mybir.ActivationFunctionType.Identity
    Exp = mybir.ActivationFunctionType.Exp
    MUL = mybir.AluOpType.mult

    with (
        tc.tile_pool(name="consts", bufs=1) as consts,
        tc.tile_pool(name="projw", bufs=1) as projw,
        tc.tile_pool(name="xtp", bufs=3) as xtp,
        tc.tile_pool(name="qkps", bufs=2, space="PSUM") as qkps,
        tc.tile_pool(name="work", bufs=2) as work,
    ):
        pjps = tc.alloc_tile_pool(name="pjps", bufs=2, space="PSUM")
        avps = None  # opened once proj psum banks are released

        # ---- constants ----
        wq_sb = projw.tile([128, NCT, 128], BF, name="wq_sb")
        wk_sb = projw.tile([128, NCT, 128], BF, name="wk_sb")
        nc.sync.dma_start(wq_sb[:, 0:1, :], wqk_d[0, :, 0:1, :])
        nc.sync.dma_start(wq_sb[:, 1:, :], wqk_d[0, :, 1:, :])
        nc.sync.dma_start(wk_sb[:], wqk_d[1])
        bq_sb = consts.tile([128, 1], F32, name="bq_sb")
        bk_sb = consts.tile([128, 1], F32, name="bk_sb")
        nc.gpsimd.dma_start(bq_sb[:], bqk_d[0].unsqueeze(1))
        nc.gpsimd.dma_start(bk_sb[:], bqk_d[1].unsqueeze(1))
        mask_sb = consts.tile([128, 128], BF, name="mask_sb")
        nc.gpsimd.dma_start(mask_sb[:], mask_d[:])

        qT2 = consts.tile([128, B * T], BF, name="qT2")
        kT2 = consts.tile([128, B * T], BF, name="kT2")
        wh_sb = consts.tile([128, HPC, CAUG], F32, name="wh_sb")
        xaug_sb = {}
        for b in range(B):
            xaug_sb[b] = consts.tile([128, NT, CAUG], BF, name=f"xaug{b}",
                                     tag="xaug", bufs=2)
        expT = {}

        def load_xaug(b, g):
            # just-in-time load of one 4-s-tile chunk of x_aug
            nc.sync.dma_start(xaug_sb[b][:, 4 * g:4 * g + 4, :],
                              xaug_d[b, :, 4 * g:4 * g + 4, :])

        def proj_tc(tci):
            xt = xtp.tile([128, NCT, TCH], BF, name=f"xt{tci}", tag="xt")
            if tci == 0:
                nc.sync.dma_start(xt[:, 0:1, :], xTt_d[tci, :, 0:1, :])
                nc.sync.dma_start(xt[:, 1:, :], xTt_d[tci, :, 1:, :])
            else:
                nc.sync.dma_start(xt[:], xTt_d[tci])
            psq = pjps.tile([128, TCH], F32, name=f"psq{tci}", tag="psq")
            psk = pjps.tile([128, TCH], F32, name=f"psk{tci}", tag="psk")
            for ct in range(NCT):
                nc.tensor.matmul(psq[:], wq_sb[:, ct, :], xt[:, ct, :],
                                 start=(ct == 0), stop=(ct == NCT - 1))
            for ct in range(NCT):
                nc.tensor.matmul(psk[:], wk_sb[:, ct, :], xt[:, ct, :],
                                 start=(ct == 0), stop=(ct == NCT - 1))
            sl = slice(tci * TCH, (tci + 1) * TCH)
            nc.vector.tensor_scalar_add(qT2[:, sl], psq[:], bq_sb[:])
            nc.vector.tensor_scalar_add(kT2[:, sl], psk[:], bk_sb[:])

        def qk_tile(b, l, j):
            # one s-tile of QK^T + exp into causal-packed storage
            if (b, l) not in expT:
                expT[(b, l)] = consts.tile([128, EXP_COLS], BF,
                                           name=f"expT{b}{l}", tag=f"expT{l}",
                                           bufs=1)
            e_ = expT[(b, l)]
            hq = qT2[l * HS:(l + 1) * HS, b * T:(b + 1) * T]
            hk = kT2[l * HS:(l + 1) * HS, b * T:(b + 1) * T]
            koff = j * 128
            kslice = hk[:, koff:koff + 128]
            for m in range(j // 4, 4):
                ps = qkps.tile([128, TCH], F32, name=f"att{b}{l}{j}{m}",
                               tag="ps512")
                if m == j // 4:
                    # variable-width first chunk: skip sub-diagonal columns
                    n0 = (m + 1) * TCH - koff
                    nc.tensor.matmul(ps[:, 0:n0], kslice,
                                     hq[:, koff:(m + 1) * TCH],
                                     start=True, stop=True)
                    dt_ = work.tile([128, 128], BF, name=f"dt{b}{l}{j}",
                                    tag="dtmp", bufs=4)
                    nc.scalar.activation(dt_[:], ps[:, 0:128], Exp,
                                         scale=0.125)
                    nc.vector.tensor_mul(out=e_[:, OFF[j]:OFF[j] + 128],
                                         in0=dt_[:], in1=mask_sb[:])
                    if n0 > 128:
                        nc.scalar.activation(
                            e_[:, OFF[j] + 128:OFF[j] + n0],
                            ps[:, 128:n0], Exp, scale=0.125)
                else:
                    nc.tensor.matmul(ps[:], kslice,
                                     hq[:, m * TCH:(m + 1) * TCH],
                                     start=True, stop=True)
                    dst = OFF[j] + m * TCH - koff
                    nc.scalar.activation(e_[:, dst:dst + TCH], ps[:], Exp,
                                         scale=0.125)

        def av_row(b, qb, l, acc):
            # AV matmuls + fused normalize/head-weight for one q-block.
            # One 3-bank psum tile; matmul chunks at bank-aligned offsets.
            ps = avps.tile([128, 3 * TCH], F32, name=f"av{b}{qb}{l}", tag="av")
            for st in range(qb + 1):
                lhsT = expT[(b, l)][:, OFF[st] + 128 * (qb - st):
                                    OFF[st] + 128 * (qb - st) + 128]
                for cc in range(3):
                    nc.tensor.matmul(ps[:, cc * TCH:cc * TCH + CCH], lhsT,
                                     xaug_sb[b][:, st, cc * CCH:(cc + 1) * CCH],
                                     start=(st == 0), stop=(st == qb))
            r_ = work.tile([128, 1], F32, name=f"r{b}{qb}{l}", tag="r", bufs=4)
            nc.vector.reciprocal(r_[:], ps[:, 2 * TCH + 340:2 * TCH + 341])
            ps3d = ps.rearrange("p (a u) -> p a u", a=3)[:, :, 0:CCH]
            nc.vector.scalar_tensor_tensor(
                out=acc.rearrange("p (a u) -> p a u", u=CCH),
                in0=ps3d, scalar=r_[:],
                in1=wh_sb[:, l, :].rearrange("p (a u) -> p a u", u=CCH),
                op0=MUL, op1=MUL)

        def fused_b(b, also_l0=False, l0_jit_from=None):
            # per q-block: finish QK (head 1, optionally head 0 too), then AV
            # for both heads, combine, store
            for qb in range(NT):
                if also_l0:
                    qk_tile(b, 0, qb)
                elif l0_jit_from is not None and l0_jit_from <= qb + 1 < NT:
                    qk_tile(b, 0, qb + 1)   # one-qb lookahead for head 0
                qk_tile(b, 1, qb)
                if qb % 4 == 2:
                    g = qb // 4 + 1
                    if b == 0:
                        if g < 4:
                            load_xaug(0, g)    # lookahead for this batch
                        load_xaug(1, g - 1)    # prefetch next batch
                    elif g < 4:
                        load_xaug(1, g)
                if b == 0 and qb == NT - 2:
                    load_xaug(1, 3)
                acc = work.tile([128, CAUG], F32, name=f"acc{b}{qb}", tag="acc",
                                bufs=2)
                tmp2 = work.tile([128, CAUG], F32, name=f"tmp2{b}{qb}",
                                 tag="tmp2", bufs=2)
                ybf = work.tile([128, C], BF, name=f"ybf{b}{qb}", tag="ybf",
                                bufs=2)
                av_row(b, qb, 0, acc)
                av_row(b, qb, 1, tmp2)
                nc.gpsimd.tensor_add(out=ybf[:], in0=acc[:, 0:C],
                                     in1=tmp2[:, 0:C])
                nc.sync.dma_start(y_d[b, qb * 128:(qb + 1) * 128, :], ybf[:])

        # ---- emission schedule ----
        def mark(name):
            MARKERS.append((name, int(nc.next_id())))

        mark("proj03")
        for tci in range(4):              # proj for batch-0 columns
            proj_tc(tci)
        mark("proj47+qk00")
        for i, tci in enumerate(range(4, NTC)):   # proj b1 cols || QK(b0,h0)
            proj_tc(tci)
            for j in range(2 * i, min(2 * i + 2, 6)):
                qk_tile(0, 0, j)
        nc.sync.dma_start(wh_sb[:], wh_d.rearrange("l p c -> p l c"))
        load_xaug(0, 0)
        pjps.release()
        avps = tc.alloc_tile_pool(name="avps", bufs=2, space="PSUM")
        mark("fused_b0")
        fused_b(0, l0_jit_from=6)
        mark("fused_b1")
        fused_b(1, also_l0=True)
        mark("end")
        avps.release()


_CACHE = {}
MARKERS = []


def _build():
    if "nc" in _CACHE:
        return _CACHE["nc"]
    nc = bacc.Bacc("TRN2", target_bir_lowering=False, debug=False,
                   enable_asserts=False, num_devices=NCORES)
    xTt_d = nc.dram_tensor("xTt", [NTC, 128, NCT, TCH], BF,
                           kind="ExternalInput").ap()
    xaug_d = nc.dram_tensor("xaug", [B, 128, NT, CAUG], BF,
                            kind="ExternalInput").ap()
    wqk_d = nc.dram_tensor("wqk", [2, 128, NCT, 128], BF,
                           kind="ExternalInput").ap()
    bqk_d = nc.dram_tensor("bqk", [2, 128], F32, kind="ExternalInput").ap()
    wh_d = nc.dram_tensor("wh", [HPC, 128, CAUG], F32, kind="ExternalInput").ap()
    mask_d = nc.dram_tensor("mask", [128, 128], BF, kind="ExternalInput").ap()
    y_d = nc.dram_tensor("y", [B, T, C], BF, kind="ExternalOutput").ap()
    with tile.TileContext(nc, trace_sim=False) as tc:
        _emit(nc, xTt_d, xaug_d, wqk_d, bqk_d, wh_d, mask_d, y_d, tc)
    nc.compile()
    _CACHE["nc"] = nc
    return nc


def _prep_inputs(x, W_attn, b_attn, head_weights):
    x = np.asarray(x, dtype=np.float32)
    W_attn = np.asarray(W_attn, dtype=np.float32)
    b_attn = np.asarray(b_attn, dtype=np.float32)
    head_weights = np.asarray(head_weights, dtype=np.float32)

    xf = x.reshape(B * T, C)
    # xTt[tc, p, ct, u] = x[tc*512+u, ct*128+p]
    xTt = np.ascontiguousarray(
        xf.reshape(NTC, TCH, NCT, 128).transpose(0, 3, 2, 1)).astype(BF16)
    xaug = np.zeros((B, T, CAUG), dtype=np.float32)
    xaug[:, :, :C] = x
    xaug[:, :, C] = 1.0
    xaug = np.ascontiguousarray(
        xaug.reshape(B, NT, 128, CAUG).transpose(0, 2, 1, 3)).astype(BF16)
    mask = np.triu(np.ones((128, 128), dtype=np.float32)).astype(BF16)

    in_maps = []
    for core in range(NCORES):
        h0 = HPC * core
        cols = np.concatenate(
            [np.arange(h * HS, (h + 1) * HS) for h in range(h0, h0 + HPC)])
        wq = W_attn[:, cols]          # [1024, 128]
        wk = W_attn[:, C + cols]
        # wqk[qk, p, ct, m] = W[ct*128+p, m]
        wqk = np.stack([
            np.ascontiguousarray(w.reshape(NCT, 128, 128).transpose(1, 0, 2))
            for w in (wq, wk)]).astype(BF16)
        bqk = np.stack([b_attn[cols], b_attn[C + cols]]).astype(np.float32)
        whp = np.zeros((HPC, CAUG), dtype=np.float32)
        whp[:, :C] = head_weights[h0:h0 + HPC]
        wh = np.ascontiguousarray(
            np.broadcast_to(whp[:, None, :], (HPC, 128, CAUG))
        ).astype(np.float32)
        in_maps.append({
            "xTt": xTt, "xaug": xaug, "mask": mask,
            "wqk": wqk, "bqk": bqk, "wh": wh,
        })
    return in_maps


def _run(inputs, trace=False, **kwargs):
    nc = _build()
    in_maps = _prep_inputs(**inputs)
    res = run_bass_kernel_spmd(nc, in_maps, core_ids=list(range(NCORES)),
                               trace=trace, **kwargs)
    y = np.zeros((B, T, C), dtype=np.float64)
    for core in range(NCORES):
        y += res.results[core]["y"].astype(np.float64)
    return y.astype(np.float32), res


def kernel(x, W_attn, b_attn, head_weights):
    y, _ = _run(dict(x=x, W_attn=W_attn, b_attn=b_attn,
                     head_weights=head_weights))
    return y



# revision 7
# speedup vs baseline: 278.5513x; 278.5513x over previous
"""Causal self-attention (shared-V, head-weighted sum) on 8 trn2 NeuronCores.

Reference computation (B=2, T=2048, C=1024, H=16, hs=64):
    qk = x @ W_attn + b_attn ; q, k = split(qk)
    att = softmax(causal(q @ k^T / sqrt(hs)))
    y   = sum_h head_weights[h] * (att_h @ x)

Sharding: tensor-parallel over heads. Core i computes heads {2i, 2i+1} for
both batches and returns its partial y; the host sums the 8 partials.

Per-core pipeline (bf16 matmuls, f32 accumulation):
  1. proj: qT/kT [128(=2 heads x 64), B*T] = W_tile^T @ x^T, bias via ACT.
  2. QK:   attT[s,q] psum chunks (512 cols), exp on ACT (scale=1/8) into
           causally-packed SBUF storage; diagonal blocks masked in-place on
           DVE. QK chunks are "dripped" one at a time between AV matmul
           groups so the tensor queue never stalls on ACT exp drains (which
           would re-throttle the PE clock via HAM).
  3. AV:   y[q,c] psum = expT^T @ x_aug where x_aug has a ones column, so the
           softmax denominator comes out of the same matmuls.
  4. norm+combine: fused DVE op (psum * 1/denom) * w_head per chunk,
           second head added on the Pool engine, DMA to DRAM.

Batch-1's first QK tiles are precomputed into a small fresh SBUF region
during batch-0's dense AV phase (the main packed planes are reused b0->b1,
so early b1 writes would otherwise serialize on b0's last reads).
"""

import numpy as np
import ml_dtypes

import concourse.bass as bass
import concourse.bacc as bacc
import concourse.mybir as mybir
import concourse.tile as tile
from concourse.bass_utils import run_bass_kernel_spmd

BF16 = ml_dtypes.bfloat16
F32 = mybir.dt.float32
BF = mybir.dt.bfloat16

B, T, C, H = 2, 2048, 1024, 16
NCORES = 8
HPC = H // NCORES          # heads per core = 2
HS = C // H                # head size = 64
NT = T // 128              # 16 s/q tiles per batch
CAUG = C + 2               # x columns + ones column + pad = 1026
CCH = CAUG // 3            # AV moving-dim chunk = 342
TCH = 512                  # proj/QK moving-dim chunk
NTC = B * T // TCH         # 8 proj t-chunks
NCT = C // 128             # 8 contraction tiles

# causally-packed expT storage: tile j holds q in [128j, 2048) -> offset table
OFF = [0] * NT
for _j in range(1, NT):
    OFF[_j] = OFF[_j - 1] + (T - 128 * (_j - 1))
EXP_COLS = OFF[NT - 1] + (T - 128 * (NT - 1))  # 17408

# batch-1 fresh-plane prefetch: j < FRESH_J tiles live in their own region
FRESH_J = 2
FBASE = [0] * FRESH_J
for _j in range(1, FRESH_J):
    FBASE[_j] = FBASE[_j - 1] + (T - 128 * (_j - 1))
FRESH_COLS = FBASE[FRESH_J - 1] + (T - 128 * (FRESH_J - 1))


def _emit(nc, xTt_d, xaug_d, wqk_d, bqk_d, wh_d, mask_d, y_d, tc):
    Exp = mybir.ActivationFunctionType.Exp
    MUL = mybir.AluOpType.mult

    with (
        tc.tile_pool(name="consts", bufs=1) as consts,
        tc.tile_pool(name="projw", bufs=1) as projw,
        tc.tile_pool(name="qkps", bufs=2, space="PSUM") as qkps,
        tc.tile_pool(name="work", bufs=2) as work,
    ):
        xtp = tc.alloc_tile_pool(name="xtp", bufs=3)
        pjps = tc.alloc_tile_pool(name="pjps", bufs=2, space="PSUM")
        avps = None   # opened once proj psum banks are released
        freshp = None

        # ---- constant tiles + priority-ordered DMA ----
        # first proj matmul needs wq[ct0] + xt0[ct0]; issue those first on
        # separate queues so compute can start ~7us earlier.
        wq_sb = projw.tile([128, NCT, 128], BF, name="wq_sb")
        wk_sb = projw.tile([128, NCT, 128], BF, name="wk_sb")
        xt0 = xtp.tile([128, NCT, TCH], BF, name="xt0", tag="xt")
        nc.sync.dma_start(wq_sb[:, 0:1, :], wqk_d[0, :, 0:1, :])
        nc.scalar.dma_start(xt0[:, 0:1, :], xTt_d[0, :, 0:1, :])
        nc.sync.dma_start(wq_sb[:, 1:, :], wqk_d[0, :, 1:, :])
        nc.scalar.dma_start(xt0[:, 1:, :], xTt_d[0, :, 1:, :])
        nc.sync.dma_start(wk_sb[:], wqk_d[1])
        bq_sb = consts.tile([128, 1], F32, name="bq_sb")
        bk_sb = consts.tile([128, 1], F32, name="bk_sb")
        nc.gpsimd.dma_start(bq_sb[:], bqk_d[0].unsqueeze(1))
        nc.gpsimd.dma_start(bk_sb[:], bqk_d[1].unsqueeze(1))
        mask_sb = consts.tile([128, 128], BF, name="mask_sb")
        nc.gpsimd.dma_start(mask_sb[:], mask_d[:])

        qT2 = consts.tile([128, B * T], BF, name="qT2")
        kT2 = consts.tile([128, B * T], BF, name="kT2")
        wh_sb = consts.tile([128, HPC, CAUG], F32, name="wh_sb")
        xaug_sb = {}
        for b in range(B):
            xaug_sb[b] = consts.tile([128, NT, CAUG], BF, name=f"xaug{b}",
                                     tag="xaug", bufs=2)
        # l -> shared packed plane (reused b0 -> b1)
        expT = {l: consts.tile([128, EXP_COLS], BF, name=f"expT{l}")
                for l in range(HPC)}
        fresh = {}  # l -> fresh plane for b1 j < FRESH_J

        def load_xaug(b, g):
            # just-in-time load of one 4-s-tile chunk of x_aug
            nc.sync.dma_start(xaug_sb[b][:, 4 * g:4 * g + 4, :],
                              xaug_d[b, :, 4 * g:4 * g + 4, :])

        # ---- QK chunk machinery ----
        # Each QK j-tile is split into <=512-col chunks. Chunks are enqueued
        # (in dependency order) and emitted one at a time between AV matmul
        # groups, so ACT exp drains overlap tensor work instead of blocking
        # the qkps psum rotation.
        chunkq = []
        remaining = {}

        def plane_for(b, j):
            if b == 1 and j < FRESH_J:
                return "fresh", FBASE[j]
            return "expT", OFF[j]

        def queue_qk(b, l, j):
            kind, base = plane_for(b, j)
            koff = j * 128
            m0 = j // 4
            remaining[(b, l, j)] = 4 - m0

            def emit_chunk(m, b=b, l=l, j=j, kind=kind, base=base, koff=koff,
                           m0=m0):
                plane = (fresh if kind == "fresh" else expT)[l]
                hq = qT2[l * HS:(l + 1) * HS, b * T:(b + 1) * T]
                hk = kT2[l * HS:(l + 1) * HS, b * T:(b + 1) * T]
                kslice = hk[:, koff:koff + 128]
                ps = qkps.tile([128, TCH], F32, name=f"qk{b}{l}{j}{m}",
                               tag="ps512")
                if m == m0:
                    n0 = (m + 1) * TCH - koff
                    nc.tensor.matmul(ps[:, 0:n0], kslice,
                                     hq[:, koff:(m + 1) * TCH],
                                     start=True, stop=True)
                    nc.scalar.activation(plane[:, base:base + n0],
                                         ps[:, 0:n0], Exp, scale=0.125)
                    nc.vector.tensor_mul(out=plane[:, base:base + 128],
                                         in0=plane[:, base:base + 128],
                                         in1=mask_sb[:])
                else:
                    dst = base + m * TCH - koff
                    nc.tensor.matmul(ps[:], kslice,
                                     hq[:, m * TCH:(m + 1) * TCH],
                                     start=True, stop=True)
                    nc.scalar.activation(plane[:, dst:dst + TCH], ps[:], Exp,
                                         scale=0.125)
                remaining[(b, l, j)] -= 1

            for m in range(m0, 4):
                chunkq.append(lambda m=m: emit_chunk(m))

        def drip(n=1):
            for _ in range(n):
                if chunkq:
                    chunkq.pop(0)()

        def need(b, l, j):
            # emit queued chunks until all of (b, l, jj<=j) are done
            def pending():
                return any(remaining.get((b, l, jj), 0) > 0
                           for jj in range(j + 1))
            while pending():
                assert chunkq, f"qk chunk ordering bug at {(b, l, j)}"
                chunkq.pop(0)()

        # ---- proj ----
        def proj_tc(tci, xt=None):
            if xt is None:
                xt = xtp.tile([128, NCT, TCH], BF, name=f"xt{tci}", tag="xt")
                nc.scalar.dma_start(xt[:], xTt_d[tci])
            psq = pjps.tile([128, TCH], F32, name=f"psq{tci}", tag="psq")
            psk = pjps.tile([128, TCH], F32, name=f"psk{tci}", tag="psk")
            for ct in range(NCT):
                nc.tensor.matmul(psq[:], wq_sb[:, ct, :], xt[:, ct, :],
                                 start=(ct == 0), stop=(ct == NCT - 1))
            drip(2)
            for ct in range(NCT):
                nc.tensor.matmul(psk[:], wk_sb[:, ct, :], xt[:, ct, :],
                                 start=(ct == 0), stop=(ct == NCT - 1))
            drip(2)
            sl = slice(tci * TCH, (tci + 1) * TCH)
            nc.vector.tensor_scalar_add(qT2[:, sl], psq[:], bq_sb[:])
            nc.vector.tensor_scalar_add(kT2[:, sl], psk[:], bk_sb[:])
            drip(1)

        # ---- AV ----
        def lhsT_slice(b, l, st, qb):
            if b == 1 and st < FRESH_J:
                off = FBASE[st] + 128 * (qb - st)
                return fresh[l][:, off:off + 128]
            off = OFF[st] + 128 * (qb - st)
            return expT[l][:, off:off + 128]

        def av_row(b, qb, l, acc):
            # AV matmuls + fused normalize/head-weight for one q-block.
            # One 3-bank psum tile; matmul chunks at bank-aligned offsets.
            # QK chunks drip in every other st group (>=2 AV MM groups of
            # spacing per chunk keeps ACT ahead of the psum rotation).
            ps = avps.tile([128, 3 * TCH], F32, name=f"av{b}{qb}{l}", tag="av")
            for st in range(qb + 1):
                lhsT = lhsT_slice(b, l, st, qb)
                for cc in range(3):
                    nc.tensor.matmul(ps[:, cc * TCH:cc * TCH + CCH], lhsT,
                                     xaug_sb[b][:, st, cc * CCH:(cc + 1) * CCH],
                                     start=(st == 0), stop=(st == qb))
                if st % 2 == 1:
                    drip(1)
            r_ = work.tile([128, 1], F32, name=f"r{b}{qb}{l}", tag="r", bufs=4)
            nc.vector.reciprocal(r_[:], ps[:, 2 * TCH + 340:2 * TCH + 341])
            ps3d = ps.rearrange("p (a u) -> p a u", a=3)[:, :, 0:CCH]
            nc.vector.scalar_tensor_tensor(
                out=acc.rearrange("p (a u) -> p a u", u=CCH),
                in0=ps3d, scalar=r_[:],
                in1=wh_sb[:, l, :].rearrange("p (a u) -> p a u", u=CCH),
                op0=MUL, op1=MUL)

        def combine_store(b, qb, acc, tmp2):
            ybf = work.tile([128, C], BF, name=f"ybf{b}{qb}", tag="ybf",
                            bufs=2)
            nc.gpsimd.tensor_add(out=ybf[:], in0=acc[:, 0:C],
                                 in1=tmp2[:, 0:C])
            nc.sync.dma_start(y_d[b, qb * 128:(qb + 1) * 128, :], ybf[:])

        def tail_row(b, qb):
            # channel-split last q-block: denominator chunk first, then the
            # other two chunks with normalize/combine/store pipelined under
            # the remaining matmuls. Head-1 normalize runs on the Pool
            # engine so the two heads' STTs overlap.
            need(b, 0, qb)
            need(b, 1, qb)
            ps = {0: avps.tile([128, 3 * TCH], F32, name="tl0", tag="av"),
                  1: avps.tile([128, 3 * TCH], F32, name="tl1", tag="av")}
            accs = {0: work.tile([128, CAUG], F32, name="tacc", tag="acc",
                                 bufs=2),
                    1: work.tile([128, CAUG], F32, name="ttmp", tag="tmp2",
                                 bufs=2)}
            ybf = work.tile([128, C], BF, name="tybf", tag="ybf", bufs=2)
            rr = {}

            def mm_chunk(l, cc):
                for st in range(qb + 1):
                    nc.tensor.matmul(
                        ps[l][:, cc * TCH:cc * TCH + CCH],
                        lhsT_slice(b, l, st, qb),
                        xaug_sb[b][:, st, cc * CCH:(cc + 1) * CCH],
                        start=(st == 0), stop=(st == qb))

            for l in (0, 1):
                mm_chunk(l, 2)
            for l in (0, 1):
                rr[l] = work.tile([128, 1], F32, name=f"tr{l}", tag="r",
                                  bufs=4)
                nc.vector.reciprocal(rr[l][:],
                                     ps[l][:, 2 * TCH + 340:2 * TCH + 341])

            def norm_combine(cc, lo, hi):
                # hi/lo are channel bounds within [cc*CCH, (cc+1)*CCH)
                w = hi - lo
                pslc = slice(cc * TCH + (lo - cc * CCH),
                             cc * TCH + (lo - cc * CCH) + w)
                # both STTs on DVE (GPSIMD has no PSUM port); add on Pool
                nc.vector.scalar_tensor_tensor(
                    out=accs[0][:, lo:hi], in0=ps[0][:, pslc],
                    scalar=rr[0][:], in1=wh_sb[:, 0, lo:hi],
                    op0=MUL, op1=MUL)
                nc.vector.scalar_tensor_tensor(
                    out=accs[1][:, lo:hi], in0=ps[1][:, pslc],
                    scalar=rr[1][:], in1=wh_sb[:, 1, lo:hi],
                    op0=MUL, op1=MUL)
                nc.gpsimd.tensor_add(out=ybf[:, lo:hi], in0=accs[0][:, lo:hi],
                                     in1=accs[1][:, lo:hi])
                nc.sync.dma_start(y_d[b, qb * 128:(qb + 1) * 128, lo:hi],
                                  ybf[:, lo:hi])

            norm_combine(2, 2 * CCH, C)     # channels 684..1023 (skip ones)
            for cc in (0, 1):
                for l in (0, 1):
                    mm_chunk(l, cc)
                norm_combine(cc, cc * CCH, (cc + 1) * CCH)

        # ---- emission schedule ----
        def mark(name):
            MARKERS.append((name, int(nc.next_id())))

        mark("proj03")
        proj_tc(0, xt=xt0)
        for tci in range(1, 4):
            proj_tc(tci)
        # b0 QK tiles that fit ACT capacity during the proj phase
        for j in range(4):
            queue_qk(0, 0, j)
            queue_qk(0, 1, j)
        queue_qk(0, 0, 4)
        queue_qk(0, 0, 5)
        mark("proj47")
        for tci in range(4, NTC):
            proj_tc(tci)
        nc.sync.dma_start(wh_sb[:], wh_d.rearrange("l p c -> p l c"))
        load_xaug(0, 0)
        xtp.release()
        pjps.release()
        avps = tc.alloc_tile_pool(name="avps", bufs=2, space="PSUM")
        freshp = tc.alloc_tile_pool(name="freshp", bufs=1)
        for l in range(HPC):
            fresh[l] = freshp.tile([128, FRESH_COLS], BF, name=f"fresh{l}")

        # rest of b0's QK, interleaved by need time (l0 row runs first)
        queue_qk(0, 1, 4)
        queue_qk(0, 1, 5)
        for j in range(6, NT):
            queue_qk(0, 0, j)
            queue_qk(0, 1, j)

        mark("fused_b0")
        for qb in range(NT):
            if qb == 12:
                # prefetch b1's first QK tiles into the fresh planes while
                # b0's AV phase is dense (no deps on the shared planes)
                for j in range(FRESH_J):
                    queue_qk(1, 0, j)
                    queue_qk(1, 1, j)
            if qb == NT - 1:
                # b1's shared-plane QK: enqueued so its chunks drip inside
                # b0's last AV row, whose early st groups release the
                # blocks these exps overwrite
                for j in range(FRESH_J, NT):
                    queue_qk(1, 0, j)
                    queue_qk(1, 1, j)
            if qb % 4 == 2:
                g = qb // 4 + 1
                if g < 4:
                    load_xaug(0, g)
                load_xaug(1, g - 1)
            if qb == NT - 2:
                load_xaug(1, 3)
            acc = work.tile([128, CAUG], F32, name=f"acc0{qb}", tag="acc",
                            bufs=2)
            tmp2 = work.tile([128, CAUG], F32, name=f"tmp20{qb}",
                             tag="tmp2", bufs=2)
            need(0, 0, qb)
            av_row(0, qb, 0, acc)
            drip(1)
            need(0, 1, qb)
            av_row(0, qb, 1, tmp2)
            combine_store(0, qb, acc, tmp2)
            drip(1)

        mark("fused_b1")
        for qb in range(NT - 1):
            acc = work.tile([128, CAUG], F32, name=f"acc1{qb}", tag="acc",
                            bufs=2)
            tmp2 = work.tile([128, CAUG], F32, name=f"tmp21{qb}",
                             tag="tmp2", bufs=2)
            need(1, 0, qb)
            av_row(1, qb, 0, acc)
            drip(1)
            need(1, 1, qb)
            av_row(1, qb, 1, tmp2)
            combine_store(1, qb, acc, tmp2)
            drip(1)
        tail_row(1, NT - 1)
        mark("end")
        avps.release()
        freshp.release()


_CACHE = {}
MARKERS = []


def _build():
    if "nc" in _CACHE:
        return _CACHE["nc"]
    nc = bacc.Bacc("TRN2", target_bir_lowering=False, debug=False,
                   enable_asserts=False, num_devices=NCORES)
    xTt_d = nc.dram_tensor("xTt", [NTC, 128, NCT, TCH], BF,
                           kind="ExternalInput").ap()
    xaug_d = nc.dram_tensor("xaug", [B, 128, NT, CAUG], BF,
                            kind="ExternalInput").ap()
    wqk_d = nc.dram_tensor("wqk", [2, 128, NCT, 128], BF,
                           kind="ExternalInput").ap()
    bqk_d = nc.dram_tensor("bqk", [2, 128], F32, kind="ExternalInput").ap()
    wh_d = nc.dram_tensor("wh", [HPC, 128, CAUG], F32, kind="ExternalInput").ap()
    mask_d = nc.dram_tensor("mask", [128, 128], BF, kind="ExternalInput").ap()
    y_d = nc.dram_tensor("y", [B, T, C], BF, kind="ExternalOutput").ap()
    with tile.TileContext(nc, trace_sim=False) as tc:
        _emit(nc, xTt_d, xaug_d, wqk_d, bqk_d, wh_d, mask_d, y_d, tc)
    nc.compile()
    _CACHE["nc"] = nc
    return nc


def _prep_inputs(x, W_attn, b_attn, head_weights):
    x = np.asarray(x, dtype=np.float32)
    W_attn = np.asarray(W_attn, dtype=np.float32)
    b_attn = np.asarray(b_attn, dtype=np.float32)
    head_weights = np.asarray(head_weights, dtype=np.float32)

    xf = x.reshape(B * T, C)
    # xTt[tc, p, ct, u] = x[tc*512+u, ct*128+p]
    xTt = np.ascontiguousarray(
        xf.reshape(NTC, TCH, NCT, 128).transpose(0, 3, 2, 1)).astype(BF16)
    xaug = np.zeros((B, T, CAUG), dtype=np.float32)
    xaug[:, :, :C] = x
    xaug[:, :, C] = 1.0
    xaug = np.ascontiguousarray(
        xaug.reshape(B, NT, 128, CAUG).transpose(0, 2, 1, 3)).astype(BF16)
    mask = np.triu(np.ones((128, 128), dtype=np.float32)).astype(BF16)

    in_maps = []
    for core in range(NCORES):
        h0 = HPC * core
        cols = np.concatenate(
            [np.arange(h * HS, (h + 1) * HS) for h in range(h0, h0 + HPC)])
        wq = W_attn[:, cols]          # [1024, 128]
        wk = W_attn[:, C + cols]
        # wqk[qk, p, ct, m] = W[ct*128+p, m]
        wqk = np.stack([
            np.ascontiguousarray(w.reshape(NCT, 128, 128).transpose(1, 0, 2))
            for w in (wq, wk)]).astype(BF16)
        bqk = np.stack([b_attn[cols], b_attn[C + cols]]).astype(np.float32)
        whp = np.zeros((HPC, CAUG), dtype=np.float32)
        whp[:, :C] = head_weights[h0:h0 + HPC]
        wh = np.ascontiguousarray(
            np.broadcast_to(whp[:, None, :], (HPC, 128, CAUG))
        ).astype(np.float32)
        in_maps.append({
            "xTt": xTt, "xaug": xaug, "mask": mask,
            "wqk": wqk, "bqk": bqk, "wh": wh,
        })
    return in_maps


def _run(inputs, trace=False, **kwargs):
    nc = _build()
    in_maps = _prep_inputs(**inputs)
    res = run_bass_kernel_spmd(nc, in_maps, core_ids=list(range(NCORES)),
                               trace=trace, **kwargs)
    y = np.zeros((B, T, C), dtype=np.float64)
    for core in range(NCORES):
        y += res.results[core]["y"].astype(np.float64)
    return y.astype(np.float32), res


def kernel(x, W_attn, b_attn, head_weights):
    y, _ = _run(dict(x=x, W_attn=W_attn, b_attn=b_attn,
                     head_weights=head_weights))
    return y


# revision 12
# speedup vs baseline: 279.5988x; 1.0038x over previous
"""Causal self-attention (shared-V, head-weighted sum) on 8 trn2 NeuronCores.

Reference computation (B=2, T=2048, C=1024, H=16, hs=64):
    qk = x @ W_attn + b_attn ; q, k = split(qk)
    att = softmax(causal(q @ k^T / sqrt(hs)))
    y   = sum_h head_weights[h] * (att_h @ x)

Sharding: tensor-parallel over heads. Core i computes heads {2i, 2i+1} for
both batches and returns its partial y; the host sums the 8 partials.

Per-core pipeline (bf16 matmuls, f32 accumulation):
  1. proj: qT/kT [128(=2 heads x 64), B*T] = W_tile^T @ x^T, bias via ACT.
  2. QK:   attT[s,q] psum chunks (512 cols), exp on ACT (scale=1/8) into
           causally-packed SBUF storage; diagonal blocks masked in-place on
           DVE. QK chunks are "dripped" one at a time between AV matmul
           groups so the tensor queue never stalls on ACT exp drains (which
           would re-throttle the PE clock via HAM).
  3. AV:   y[q,c] psum = expT^T @ x_aug where x_aug has a ones column, so the
           softmax denominator comes out of the same matmuls.
  4. norm+combine: fused DVE op (psum * 1/denom) * w_head per chunk,
           second head added on the Pool engine, DMA to DRAM.

Batch-1's first QK tiles are precomputed into a small fresh SBUF region
during batch-0's dense AV phase (the main packed planes are reused b0->b1,
so early b1 writes would otherwise serialize on b0's last reads).
"""

import numpy as np
import ml_dtypes

import concourse.bass as bass
import concourse.bacc as bacc
import concourse.mybir as mybir
import concourse.tile as tile
from concourse.bass_utils import run_bass_kernel_spmd

BF16 = ml_dtypes.bfloat16
F32 = mybir.dt.float32
BF = mybir.dt.bfloat16

B, T, C, H = 2, 2048, 1024, 16
NCORES = 8
HPC = H // NCORES          # heads per core = 2
HS = C // H                # head size = 64
NT = T // 128              # 16 s/q tiles per batch
CAUG = C + 2               # x columns + ones column + pad = 1026
CCH = CAUG // 3            # AV moving-dim chunk = 342
TCH = 512                  # proj/QK moving-dim chunk
NTC = B * T // TCH         # 8 proj t-chunks
NCT = C // 128             # 8 contraction tiles

# causally-packed expT storage: tile j holds q in [128j, 2048) -> offset table
OFF = [0] * NT
for _j in range(1, NT):
    OFF[_j] = OFF[_j - 1] + (T - 128 * (_j - 1))
EXP_COLS = OFF[NT - 1] + (T - 128 * (NT - 1))  # 17408

# batch-1 fresh-plane prefetch: j < FRESH_J tiles live in their own region
FRESH_J = 2
FBASE = [0] * FRESH_J
for _j in range(1, FRESH_J):
    FBASE[_j] = FBASE[_j - 1] + (T - 128 * (_j - 1))
FRESH_COLS = FBASE[FRESH_J - 1] + (T - 128 * (FRESH_J - 1))


def _emit(nc, xTt_d, xaug_d, wqk_d, bqk_d, wh_d, mask_d, y_d, tc):
    Exp = mybir.ActivationFunctionType.Exp
    MUL = mybir.AluOpType.mult

    with (
        tc.tile_pool(name="consts", bufs=1) as consts,
        tc.tile_pool(name="projw", bufs=1) as projw,
        tc.tile_pool(name="qkps", bufs=2, space="PSUM") as qkps,
        tc.tile_pool(name="work", bufs=2) as work,
    ):
        xtp = tc.alloc_tile_pool(name="xtp", bufs=3)
        pjps = tc.alloc_tile_pool(name="pjps", bufs=2, space="PSUM")
        avps = None   # opened once proj psum banks are released
        freshp = None

        # ---- constant tiles + priority-ordered DMA ----
        # first proj matmul needs wq[ct0] + xt0[ct0]; issue those first on
        # separate queues so compute can start ~7us earlier.
        wq_sb = projw.tile([128, NCT, 128], BF, name="wq_sb")
        wk_sb = projw.tile([128, NCT, 128], BF, name="wk_sb")
        xt0 = xtp.tile([128, NCT, TCH], BF, name="xt0", tag="xt")
        nc.sync.dma_start(wq_sb[:, 0:1, :], wqk_d[0, :, 0:1, :])
        nc.gpsimd.dma_start(xt0[:, 0:1, :], xTt_d[0, :, 0:1, :])
        nc.sync.dma_start(wq_sb[:, 1:, :], wqk_d[0, :, 1:, :])
        nc.gpsimd.dma_start(xt0[:, 1:, :], xTt_d[0, :, 1:, :])
        nc.sync.dma_start(wk_sb[:], wqk_d[1])
        bq_sb = consts.tile([128, 1], F32, name="bq_sb")
        bk_sb = consts.tile([128, 1], F32, name="bk_sb")
        nc.gpsimd.dma_start(bq_sb[:], bqk_d[0].unsqueeze(1))
        nc.gpsimd.dma_start(bk_sb[:], bqk_d[1].unsqueeze(1))
        mask_sb = consts.tile([128, 128], BF, name="mask_sb")
        nc.gpsimd.dma_start(mask_sb[:], mask_d[:])

        qT2 = consts.tile([128, B * T], BF, name="qT2")
        kT2 = consts.tile([128, B * T], BF, name="kT2")
        wh_sb = consts.tile([128, HPC, CAUG], F32, name="wh_sb")
        xaug_sb = {}
        for b in range(B):
            xaug_sb[b] = consts.tile([128, NT, CAUG], BF, name=f"xaug{b}",
                                     tag="xaug", bufs=2)
        # l -> shared packed plane (reused b0 -> b1)
        expT = {l: consts.tile([128, EXP_COLS], BF, name=f"expT{l}")
                for l in range(HPC)}
        fresh = {}  # l -> fresh plane for b1 j < FRESH_J

        def load_xaug(b, g):
            # just-in-time load of one 4-s-tile chunk of x_aug
            nc.sync.dma_start(xaug_sb[b][:, 4 * g:4 * g + 4, :],
                              xaug_d[b, :, 4 * g:4 * g + 4, :])

        # ---- QK chunk machinery ----
        # Each QK j-tile is split into <=512-col chunks. Chunks are enqueued
        # (in dependency order) and emitted one at a time between AV matmul
        # groups, so ACT exp drains overlap tensor work instead of blocking
        # the qkps psum rotation.
        chunkq = []
        remaining = {}

        def plane_for(b, j):
            if b == 1 and j < FRESH_J:
                return "fresh", FBASE[j]
            return "expT", OFF[j]

        def queue_qk(b, l, j):
            kind, base = plane_for(b, j)
            koff = j * 128
            m0 = j // 4
            remaining[(b, l, j)] = 4 - m0

            def emit_chunk(m, b=b, l=l, j=j, kind=kind, base=base, koff=koff,
                           m0=m0):
                plane = (fresh if kind == "fresh" else expT)[l]
                hq = qT2[l * HS:(l + 1) * HS, b * T:(b + 1) * T]
                hk = kT2[l * HS:(l + 1) * HS, b * T:(b + 1) * T]
                kslice = hk[:, koff:koff + 128]
                ps = qkps.tile([128, TCH], F32, name=f"qk{b}{l}{j}{m}",
                               tag="ps512")
                if m == m0:
                    n0 = (m + 1) * TCH - koff
                    nc.tensor.matmul(ps[:, 0:n0], kslice,
                                     hq[:, koff:(m + 1) * TCH],
                                     start=True, stop=True)
                    nc.scalar.activation(plane[:, base:base + n0],
                                         ps[:, 0:n0], Exp, scale=0.125)
                    nc.vector.tensor_mul(out=plane[:, base:base + 128],
                                         in0=plane[:, base:base + 128],
                                         in1=mask_sb[:])
                else:
                    dst = base + m * TCH - koff
                    nc.tensor.matmul(ps[:], kslice,
                                     hq[:, m * TCH:(m + 1) * TCH],
                                     start=True, stop=True)
                    nc.scalar.activation(plane[:, dst:dst + TCH], ps[:], Exp,
                                         scale=0.125)
                remaining[(b, l, j)] -= 1

            for m in range(m0, 4):
                chunkq.append(lambda m=m: emit_chunk(m))

        def drip(n=1):
            for _ in range(n):
                if chunkq:
                    chunkq.pop(0)()

        def need(b, l, j):
            # emit queued chunks until all of (b, l, jj<=j) are done
            def pending():
                return any(remaining.get((b, l, jj), 0) > 0
                           for jj in range(j + 1))
            while pending():
                assert chunkq, f"qk chunk ordering bug at {(b, l, j)}"
                chunkq.pop(0)()

        # ---- proj ----
        def proj_tc(tci, xt=None):
            if xt is None:
                # gpsimd queue (idle during proj): a trigger on the scalar
                # queue would sit behind dripped exps and starve proj of xt
                xt = xtp.tile([128, NCT, TCH], BF, name=f"xt{tci}", tag="xt")
                nc.gpsimd.dma_start(xt[:], xTt_d[tci])
            psq = pjps.tile([128, TCH], F32, name=f"psq{tci}", tag="psq")
            psk = pjps.tile([128, TCH], F32, name=f"psk{tci}", tag="psk")
            for ct in range(NCT):
                nc.tensor.matmul(psq[:], wq_sb[:, ct, :], xt[:, ct, :],
                                 start=(ct == 0), stop=(ct == NCT - 1))
            drip(2)
            for ct in range(NCT):
                nc.tensor.matmul(psk[:], wk_sb[:, ct, :], xt[:, ct, :],
                                 start=(ct == 0), stop=(ct == NCT - 1))
            drip(2)
            sl = slice(tci * TCH, (tci + 1) * TCH)
            nc.vector.tensor_scalar_add(qT2[:, sl], psq[:], bq_sb[:])
            nc.vector.tensor_scalar_add(kT2[:, sl], psk[:], bk_sb[:])
            drip(1)

        # ---- AV ----
        def lhsT_slice(b, l, st, qb):
            if b == 1 and st < FRESH_J:
                off = FBASE[st] + 128 * (qb - st)
                return fresh[l][:, off:off + 128]
            off = OFF[st] + 128 * (qb - st)
            return expT[l][:, off:off + 128]

        def av_row(b, qb, l, acc):
            # AV matmuls + fused normalize/head-weight for one q-block.
            # One 3-bank psum tile; matmul chunks at bank-aligned offsets.
            # QK chunks drip in every other st group (>=2 AV MM groups of
            # spacing per chunk keeps ACT ahead of the psum rotation).
            ps = avps.tile([128, 3 * TCH], F32, name=f"av{b}{qb}{l}", tag="av")
            for st in range(qb + 1):
                lhsT = lhsT_slice(b, l, st, qb)
                for cc in range(3):
                    nc.tensor.matmul(ps[:, cc * TCH:cc * TCH + CCH], lhsT,
                                     xaug_sb[b][:, st, cc * CCH:(cc + 1) * CCH],
                                     start=(st == 0), stop=(st == qb))
                if st % 2 == 1:
                    drip(1)
            r_ = work.tile([128, 1], F32, name=f"r{b}{qb}{l}", tag="r", bufs=4)
            nc.vector.reciprocal(r_[:], ps[:, 2 * TCH + 340:2 * TCH + 341])
            ps3d = ps.rearrange("p (a u) -> p a u", a=3)[:, :, 0:CCH]
            nc.vector.scalar_tensor_tensor(
                out=acc.rearrange("p (a u) -> p a u", u=CCH),
                in0=ps3d, scalar=r_[:],
                in1=wh_sb[:, l, :].rearrange("p (a u) -> p a u", u=CCH),
                op0=MUL, op1=MUL)

        def combine_store(b, qb, acc, tmp2):
            ybf = work.tile([128, C], BF, name=f"ybf{b}{qb}", tag="ybf",
                            bufs=2)
            nc.gpsimd.tensor_add(out=ybf[:], in0=acc[:, 0:C],
                                 in1=tmp2[:, 0:C])
            nc.sync.dma_start(y_d[b, qb * 128:(qb + 1) * 128, :], ybf[:])

        def tail_row(b, qb):
            # channel-split last q-block: denominator chunk first, then the
            # other two chunks with normalize/combine/store pipelined under
            # the remaining matmuls. Head-1 normalize runs on the Pool
            # engine so the two heads' STTs overlap.
            need(b, 0, qb)
            need(b, 1, qb)
            ps = {0: avps.tile([128, 3 * TCH], F32, name="tl0", tag="av"),
                  1: avps.tile([128, 3 * TCH], F32, name="tl1", tag="av")}
            accs = {0: work.tile([128, CAUG], F32, name="tacc", tag="acc",
                                 bufs=2),
                    1: work.tile([128, CAUG], F32, name="ttmp", tag="tmp2",
                                 bufs=2)}
            ybf = work.tile([128, C], BF, name="tybf", tag="ybf", bufs=2)
            rr = {}

            def mm_chunk(l, cc):
                for st in range(qb + 1):
                    nc.tensor.matmul(
                        ps[l][:, cc * TCH:cc * TCH + CCH],
                        lhsT_slice(b, l, st, qb),
                        xaug_sb[b][:, st, cc * CCH:(cc + 1) * CCH],
                        start=(st == 0), stop=(st == qb))

            for l in (0, 1):
                mm_chunk(l, 2)
            for l in (0, 1):
                rr[l] = work.tile([128, 1], F32, name=f"tr{l}", tag="r",
                                  bufs=4)
                nc.vector.reciprocal(rr[l][:],
                                     ps[l][:, 2 * TCH + 340:2 * TCH + 341])

            def norm_combine(cc, lo, hi):
                # hi/lo are channel bounds within [cc*CCH, (cc+1)*CCH)
                w = hi - lo
                pslc = slice(cc * TCH + (lo - cc * CCH),
                             cc * TCH + (lo - cc * CCH) + w)
                # both STTs on DVE (GPSIMD has no PSUM port); add on Pool
                nc.vector.scalar_tensor_tensor(
                    out=accs[0][:, lo:hi], in0=ps[0][:, pslc],
                    scalar=rr[0][:], in1=wh_sb[:, 0, lo:hi],
                    op0=MUL, op1=MUL)
                nc.vector.scalar_tensor_tensor(
                    out=accs[1][:, lo:hi], in0=ps[1][:, pslc],
                    scalar=rr[1][:], in1=wh_sb[:, 1, lo:hi],
                    op0=MUL, op1=MUL)
                nc.gpsimd.tensor_add(out=ybf[:, lo:hi], in0=accs[0][:, lo:hi],
                                     in1=accs[1][:, lo:hi])
                nc.sync.dma_start(y_d[b, qb * 128:(qb + 1) * 128, lo:hi],
                                  ybf[:, lo:hi])

            norm_combine(2, 2 * CCH, C)     # channels 684..1023 (skip ones)
            for cc in (0, 1):
                for l in (0, 1):
                    mm_chunk(l, cc)
                norm_combine(cc, cc * CCH, (cc + 1) * CCH)

        # ---- emission schedule ----
        def mark(name):
            MARKERS.append((name, int(nc.next_id())))

        mark("proj03")
        proj_tc(0, xt=xt0)
        for tci in range(1, 4):
            proj_tc(tci)
        # b0 QK tiles that fit ACT capacity during the proj phase
        for j in range(4):
            queue_qk(0, 0, j)
            queue_qk(0, 1, j)
        queue_qk(0, 0, 4)
        queue_qk(0, 0, 5)
        mark("proj47")
        for tci in range(4, NTC):
            proj_tc(tci)
        nc.sync.dma_start(wh_sb[:], wh_d.rearrange("l p c -> p l c"))
        load_xaug(0, 0)
        xtp.release()
        pjps.release()
        avps = tc.alloc_tile_pool(name="avps", bufs=2, space="PSUM")
        freshp = tc.alloc_tile_pool(name="freshp", bufs=1)
        for l in range(HPC):
            fresh[l] = freshp.tile([128, FRESH_COLS], BF, name=f"fresh{l}")

        # rest of b0's QK, interleaved by need time (l0 row runs first)
        queue_qk(0, 1, 4)
        queue_qk(0, 1, 5)
        for j in range(6, NT):
            queue_qk(0, 0, j)
            queue_qk(0, 1, j)

        mark("fused_b0")
        for qb in range(NT):
            if qb == 12:
                # prefetch b1's first QK tiles into the fresh planes while
                # b0's AV phase is dense (no deps on the shared planes)
                for j in range(FRESH_J):
                    queue_qk(1, 0, j)
                    queue_qk(1, 1, j)
            if qb == NT - 1:
                # b1's first shared-plane head-0 QK tiles: their chunks drip
                # inside b0's last head-0 AV row. A chunk of tile j popped at
                # st-slot s only overwrites blocks whose b0 readers were
                # emitted at st<=j < s, so per-slice WAR deps stay correct.
                # Head-1 tiles must NOT be enqueued yet: this row's head-1
                # reader is emitted after the head-0 row.
                queue_qk(1, 0, FRESH_J)
                queue_qk(1, 0, FRESH_J + 1)
            if qb % 4 == 2:
                g = qb // 4 + 1
                if g < 4:
                    load_xaug(0, g)
                load_xaug(1, g - 1)
            if qb == NT - 2:
                load_xaug(1, 3)
            acc = work.tile([128, CAUG], F32, name=f"acc0{qb}", tag="acc",
                            bufs=2)
            tmp2 = work.tile([128, CAUG], F32, name=f"tmp20{qb}",
                             tag="tmp2", bufs=2)
            need(0, 0, qb)
            av_row(0, qb, 0, acc)
            drip(1)
            if qb == NT - 1:
                # now that the head-0 row is emitted, b1's head-1 tiles may
                # drip inside the head-1 row (same slot-after-reader rule);
                # the rest interleaves by need time in b1's loop
                queue_qk(1, 1, FRESH_J)
                queue_qk(1, 1, FRESH_J + 1)
                for j in range(FRESH_J + 2, NT):
                    queue_qk(1, 0, j)
                    queue_qk(1, 1, j)
            need(0, 1, qb)
            av_row(0, qb, 1, tmp2)
            combine_store(0, qb, acc, tmp2)
            drip(1)

        mark("fused_b1")
        for qb in range(NT - 1):
            acc = work.tile([128, CAUG], F32, name=f"acc1{qb}", tag="acc",
                            bufs=2)
            tmp2 = work.tile([128, CAUG], F32, name=f"tmp21{qb}",
                             tag="tmp2", bufs=2)
            need(1, 0, qb)
            av_row(1, qb, 0, acc)
            drip(1)
            need(1, 1, qb)
            av_row(1, qb, 1, tmp2)
            combine_store(1, qb, acc, tmp2)
            drip(1)
        tail_row(1, NT - 1)
        mark("end")
        avps.release()
        freshp.release()


_CACHE = {}
MARKERS = []


def _build():
    if "nc" in _CACHE:
        return _CACHE["nc"]
    nc = bacc.Bacc("TRN2", target_bir_lowering=False, debug=False,
                   enable_asserts=False, num_devices=NCORES)
    xTt_d = nc.dram_tensor("xTt", [NTC, 128, NCT, TCH], BF,
                           kind="ExternalInput").ap()
    xaug_d = nc.dram_tensor("xaug", [B, 128, NT, CAUG], BF,
                            kind="ExternalInput").ap()
    wqk_d = nc.dram_tensor("wqk", [2, 128, NCT, 128], BF,
                           kind="ExternalInput").ap()
    bqk_d = nc.dram_tensor("bqk", [2, 128], F32, kind="ExternalInput").ap()
    wh_d = nc.dram_tensor("wh", [HPC, 128, CAUG], F32, kind="ExternalInput").ap()
    mask_d = nc.dram_tensor("mask", [128, 128], BF, kind="ExternalInput").ap()
    y_d = nc.dram_tensor("y", [B, T, C], BF, kind="ExternalOutput").ap()
    with tile.TileContext(nc, trace_sim=False) as tc:
        _emit(nc, xTt_d, xaug_d, wqk_d, bqk_d, wh_d, mask_d, y_d, tc)
    nc.compile()
    _CACHE["nc"] = nc
    return nc


def _prep_inputs(x, W_attn, b_attn, head_weights):
    x = np.asarray(x, dtype=np.float32)
    W_attn = np.asarray(W_attn, dtype=np.float32)
    b_attn = np.asarray(b_attn, dtype=np.float32)
    head_weights = np.asarray(head_weights, dtype=np.float32)

    xf = x.reshape(B * T, C)
    # xTt[tc, p, ct, u] = x[tc*512+u, ct*128+p]
    xTt = np.ascontiguousarray(
        xf.reshape(NTC, TCH, NCT, 128).transpose(0, 3, 2, 1)).astype(BF16)
    xaug = np.zeros((B, T, CAUG), dtype=np.float32)
    xaug[:, :, :C] = x
    xaug[:, :, C] = 1.0
    xaug = np.ascontiguousarray(
        xaug.reshape(B, NT, 128, CAUG).transpose(0, 2, 1, 3)).astype(BF16)
    mask = np.triu(np.ones((128, 128), dtype=np.float32)).astype(BF16)

    in_maps = []
    for core in range(NCORES):
        h0 = HPC * core
        cols = np.concatenate(
            [np.arange(h * HS, (h + 1) * HS) for h in range(h0, h0 + HPC)])
        wq = W_attn[:, cols]          # [1024, 128]
        wk = W_attn[:, C + cols]
        # wqk[qk, p, ct, m] = W[ct*128+p, m]
        wqk = np.stack([
            np.ascontiguousarray(w.reshape(NCT, 128, 128).transpose(1, 0, 2))
            for w in (wq, wk)]).astype(BF16)
        bqk = np.stack([b_attn[cols], b_attn[C + cols]]).astype(np.float32)
        whp = np.zeros((HPC, CAUG), dtype=np.float32)
        whp[:, :C] = head_weights[h0:h0 + HPC]
        wh = np.ascontiguousarray(
            np.broadcast_to(whp[:, None, :], (HPC, 128, CAUG))
        ).astype(np.float32)
        in_maps.append({
            "xTt": xTt, "xaug": xaug, "mask": mask,
            "wqk": wqk, "bqk": bqk, "wh": wh,
        })
    return in_maps


def _run(inputs, trace=False, **kwargs):
    nc = _build()
    in_maps = _prep_inputs(**inputs)
    res = run_bass_kernel_spmd(nc, in_maps, core_ids=list(range(NCORES)),
                               trace=trace, **kwargs)
    y = np.zeros((B, T, C), dtype=np.float64)
    for core in range(NCORES):
        y += res.results[core]["y"].astype(np.float64)
    return y.astype(np.float32), res


def kernel(x, W_attn, b_attn, head_weights):
    y, _ = _run(dict(x=x, W_attn=W_attn, b_attn=b_attn,
                     head_weights=head_weights))
    return y


# revision 13
# speedup vs baseline: 296.1960x; 1.0594x over previous
"""Causal self-attention (shared-V, head-weighted sum) on 8 trn2 NeuronCores.

Reference computation (B=2, T=2048, C=1024, H=16, hs=64):
    qk = x @ W_attn + b_attn ; q, k = split(qk)
    att = softmax(causal(q @ k^T / sqrt(hs)))
    y   = sum_h head_weights[h] * (att_h @ x)

Sharding: tensor-parallel over heads. Core i computes heads {2i, 2i+1} for
both batches and returns its partial y; the host sums the 8 partials.

Per-core pipeline (bf16 matmuls, f32 accumulation):
  1. proj: qT/kT [128(=2 heads x 64), B*T] = W_tile^T @ x^T, bias via ACT.
  2. QK:   attT[s,q] psum chunks (512 cols), exp on ACT (scale=1/8) into
           causally-packed SBUF storage; diagonal blocks masked in-place on
           DVE. QK chunks are "dripped" one at a time between AV matmul
           groups so the tensor queue never stalls on ACT exp drains (which
           would re-throttle the PE clock via HAM).
  3. AV:   y[q,c] psum = expT^T @ x_aug where x_aug has a ones column, so the
           softmax denominator comes out of the same matmuls.
  4. norm+combine: fused DVE op (psum * 1/denom) * w_head per chunk,
           second head added on the Pool engine, DMA to DRAM.

Batch-1's first QK tiles are precomputed into a small fresh SBUF region
during batch-0's dense AV phase (the main packed planes are reused b0->b1,
so early b1 writes would otherwise serialize on b0's last reads).
"""

import numpy as np
import ml_dtypes

import concourse.bass as bass
import concourse.bacc as bacc
import concourse.mybir as mybir
import concourse.tile as tile
from concourse.bass_utils import run_bass_kernel_spmd

BF16 = ml_dtypes.bfloat16
F32 = mybir.dt.float32
BF = mybir.dt.bfloat16

B, T, C, H = 2, 2048, 1024, 16
NCORES = 8
HPC = H // NCORES          # heads per core = 2
HS = C // H                # head size = 64
NT = T // 128              # 16 s/q tiles per batch
CAUG = C + 2               # x columns + ones column + pad = 1026
CCH = CAUG // 3            # AV moving-dim chunk = 342
TCH = 512                  # proj/QK moving-dim chunk
NTC = B * T // TCH         # 8 proj t-chunks
NCT = C // 128             # 8 contraction tiles

# causally-packed expT storage: tile j holds q in [128j, 2048) -> offset table
OFF = [0] * NT
for _j in range(1, NT):
    OFF[_j] = OFF[_j - 1] + (T - 128 * (_j - 1))
EXP_COLS = OFF[NT - 1] + (T - 128 * (NT - 1))  # 17408

# batch-1 fresh-plane prefetch: j < FRESH_J tiles live in their own region
FRESH_J = 2
FBASE = [0] * FRESH_J
for _j in range(1, FRESH_J):
    FBASE[_j] = FBASE[_j - 1] + (T - 128 * (_j - 1))
FRESH_COLS = FBASE[FRESH_J - 1] + (T - 128 * (FRESH_J - 1))


def _emit(nc, xTt_d, xaug_d, wqk_d, bqk_d, wh_d, mask_d, y_d, tc):
    Exp = mybir.ActivationFunctionType.Exp
    MUL = mybir.AluOpType.mult

    with (
        tc.tile_pool(name="consts", bufs=1) as consts,
        tc.tile_pool(name="projw", bufs=1) as projw,
        tc.tile_pool(name="qkps", bufs=2, space="PSUM") as qkps,
        tc.tile_pool(name="work", bufs=2) as work,
    ):
        xtp = tc.alloc_tile_pool(name="xtp", bufs=3)
        pjps = tc.alloc_tile_pool(name="pjps", bufs=2, space="PSUM")
        avps = None   # opened once proj psum banks are released
        freshp = None

        # ---- constant tiles + priority-ordered DMA ----
        # first proj matmul needs wq[ct0] + xt0[ct0]; issue those first on
        # separate queues so compute can start ~7us earlier.
        wq_sb = projw.tile([128, NCT, 128], BF, name="wq_sb")
        wk_sb = projw.tile([128, NCT, 128], BF, name="wk_sb")
        xt0 = xtp.tile([128, NCT, TCH], BF, name="xt0", tag="xt")
        nc.sync.dma_start(wq_sb[:, 0:1, :], wqk_d[0, :, 0:1, :])
        nc.sync.dma_start(xt0[:, 0:1, :], xTt_d[0, :, 0:1, :])
        nc.sync.dma_start(wq_sb[:, 1:, :], wqk_d[0, :, 1:, :])
        nc.sync.dma_start(xt0[:, 1:, :], xTt_d[0, :, 1:, :])
        nc.sync.dma_start(wk_sb[:], wqk_d[1])
        bq_sb = consts.tile([128, 1], F32, name="bq_sb")
        bk_sb = consts.tile([128, 1], F32, name="bk_sb")
        nc.gpsimd.dma_start(bq_sb[:], bqk_d[0].unsqueeze(1))
        nc.gpsimd.dma_start(bk_sb[:], bqk_d[1].unsqueeze(1))
        mask_sb = consts.tile([128, 128], BF, name="mask_sb")
        nc.gpsimd.dma_start(mask_sb[:], mask_d[:])

        qT2 = consts.tile([128, B * T], BF, name="qT2")
        kT2 = consts.tile([128, B * T], BF, name="kT2")
        wh_sb = consts.tile([128, HPC, CAUG], F32, name="wh_sb")
        xaug_sb = {}
        for b in range(B):
            xaug_sb[b] = consts.tile([128, NT, CAUG], BF, name=f"xaug{b}",
                                     tag="xaug", bufs=2)
        # l -> shared packed plane (reused b0 -> b1)
        expT = {l: consts.tile([128, EXP_COLS], BF, name=f"expT{l}")
                for l in range(HPC)}
        fresh = {}  # l -> fresh plane for b1 j < FRESH_J

        def load_xaug(b, g):
            # just-in-time load of one 4-s-tile chunk of x_aug
            nc.sync.dma_start(xaug_sb[b][:, 4 * g:4 * g + 4, :],
                              xaug_d[b, :, 4 * g:4 * g + 4, :])

        # ---- QK chunk machinery ----
        # Each QK j-tile is split into <=512-col chunks. Chunks are enqueued
        # (in dependency order) and emitted one at a time between AV matmul
        # groups, so ACT exp drains overlap tensor work instead of blocking
        # the qkps psum rotation.
        chunkq = []
        remaining = {}

        def plane_for(b, j):
            if b == 1 and j < FRESH_J:
                return "fresh", FBASE[j]
            return "expT", OFF[j]

        def queue_qk(b, l, j):
            kind, base = plane_for(b, j)
            koff = j * 128
            m0 = j // 4
            remaining[(b, l, j)] = 4 - m0

            def emit_chunk(m, b=b, l=l, j=j, kind=kind, base=base, koff=koff,
                           m0=m0):
                plane = (fresh if kind == "fresh" else expT)[l]
                hq = qT2[l * HS:(l + 1) * HS, b * T:(b + 1) * T]
                hk = kT2[l * HS:(l + 1) * HS, b * T:(b + 1) * T]
                kslice = hk[:, koff:koff + 128]
                ps = qkps.tile([128, TCH], F32, name=f"qk{b}{l}{j}{m}",
                               tag="ps512")
                if m == m0:
                    n0 = (m + 1) * TCH - koff
                    nc.tensor.matmul(ps[:, 0:n0], kslice,
                                     hq[:, koff:(m + 1) * TCH],
                                     start=True, stop=True)
                    nc.scalar.activation(plane[:, base:base + n0],
                                         ps[:, 0:n0], Exp, scale=0.125)
                    nc.vector.tensor_mul(out=plane[:, base:base + 128],
                                         in0=plane[:, base:base + 128],
                                         in1=mask_sb[:])
                else:
                    dst = base + m * TCH - koff
                    nc.tensor.matmul(ps[:], kslice,
                                     hq[:, m * TCH:(m + 1) * TCH],
                                     start=True, stop=True)
                    nc.scalar.activation(plane[:, dst:dst + TCH], ps[:], Exp,
                                         scale=0.125)
                remaining[(b, l, j)] -= 1

            for m in range(m0, 4):
                chunkq.append(lambda m=m: emit_chunk(m))

        def drip(n=1):
            for _ in range(n):
                if chunkq:
                    chunkq.pop(0)()

        def need(b, l, j):
            # emit queued chunks until all of (b, l, jj<=j) are done
            def pending():
                return any(remaining.get((b, l, jj), 0) > 0
                           for jj in range(j + 1))
            while pending():
                assert chunkq, f"qk chunk ordering bug at {(b, l, j)}"
                chunkq.pop(0)()

        # ---- proj ----
        def proj_tc(tci, xt=None):
            if xt is None:
                # sync queue (hardware DGE; idle during proj) - scalar would
                # block behind dripped exps, gpsimd is the slow software path
                xt = xtp.tile([128, NCT, TCH], BF, name=f"xt{tci}", tag="xt")
                nc.sync.dma_start(xt[:], xTt_d[tci])
            psq = pjps.tile([128, TCH], F32, name=f"psq{tci}", tag="psq")
            psk = pjps.tile([128, TCH], F32, name=f"psk{tci}", tag="psk")
            for ct in range(NCT):
                nc.tensor.matmul(psq[:], wq_sb[:, ct, :], xt[:, ct, :],
                                 start=(ct == 0), stop=(ct == NCT - 1))
            drip(2)
            for ct in range(NCT):
                nc.tensor.matmul(psk[:], wk_sb[:, ct, :], xt[:, ct, :],
                                 start=(ct == 0), stop=(ct == NCT - 1))
            drip(2)
            sl = slice(tci * TCH, (tci + 1) * TCH)
            nc.vector.tensor_scalar_add(qT2[:, sl], psq[:], bq_sb[:])
            nc.vector.tensor_scalar_add(kT2[:, sl], psk[:], bk_sb[:])
            drip(1)

        # ---- AV ----
        def lhsT_slice(b, l, st, qb):
            if b == 1 and st < FRESH_J:
                off = FBASE[st] + 128 * (qb - st)
                return fresh[l][:, off:off + 128]
            off = OFF[st] + 128 * (qb - st)
            return expT[l][:, off:off + 128]

        def av_row(b, qb, l, acc):
            # AV matmuls + fused normalize/head-weight for one q-block.
            # One 3-bank psum tile; matmul chunks at bank-aligned offsets.
            # QK chunks drip in every other st group (>=2 AV MM groups of
            # spacing per chunk keeps ACT ahead of the psum rotation).
            ps = avps.tile([128, 3 * TCH], F32, name=f"av{b}{qb}{l}", tag="av")
            for st in range(qb + 1):
                lhsT = lhsT_slice(b, l, st, qb)
                for cc in range(3):
                    nc.tensor.matmul(ps[:, cc * TCH:cc * TCH + CCH], lhsT,
                                     xaug_sb[b][:, st, cc * CCH:(cc + 1) * CCH],
                                     start=(st == 0), stop=(st == qb))
                if st % 2 == 1:
                    drip(1)
            r_ = work.tile([128, 1], F32, name=f"r{b}{qb}{l}", tag="r", bufs=4)
            nc.vector.reciprocal(r_[:], ps[:, 2 * TCH + 340:2 * TCH + 341])
            ps3d = ps.rearrange("p (a u) -> p a u", a=3)[:, :, 0:CCH]
            nc.vector.scalar_tensor_tensor(
                out=acc.rearrange("p (a u) -> p a u", u=CCH),
                in0=ps3d, scalar=r_[:],
                in1=wh_sb[:, l, :].rearrange("p (a u) -> p a u", u=CCH),
                op0=MUL, op1=MUL)

        def combine_store(b, qb, acc, tmp2):
            ybf = work.tile([128, C], BF, name=f"ybf{b}{qb}", tag="ybf",
                            bufs=2)
            nc.gpsimd.tensor_add(out=ybf[:], in0=acc[:, 0:C],
                                 in1=tmp2[:, 0:C])
            nc.sync.dma_start(y_d[b, qb * 128:(qb + 1) * 128, :], ybf[:])

        def tail_row(b, qb):
            # channel-split last q-block: denominator chunk first, then the
            # other two chunks with normalize/combine/store pipelined under
            # the remaining matmuls. Head-1 normalize runs on the Pool
            # engine so the two heads' STTs overlap.
            need(b, 0, qb)
            need(b, 1, qb)
            ps = {0: avps.tile([128, 3 * TCH], F32, name="tl0", tag="av"),
                  1: avps.tile([128, 3 * TCH], F32, name="tl1", tag="av")}
            accs = {0: work.tile([128, CAUG], F32, name="tacc", tag="acc",
                                 bufs=2),
                    1: work.tile([128, CAUG], F32, name="ttmp", tag="tmp2",
                                 bufs=2)}
            ybf = work.tile([128, C], BF, name="tybf", tag="ybf", bufs=2)
            rr = {}

            def mm_chunk(l, cc):
                for st in range(qb + 1):
                    nc.tensor.matmul(
                        ps[l][:, cc * TCH:cc * TCH + CCH],
                        lhsT_slice(b, l, st, qb),
                        xaug_sb[b][:, st, cc * CCH:(cc + 1) * CCH],
                        start=(st == 0), stop=(st == qb))

            for l in (0, 1):
                mm_chunk(l, 2)
            for l in (0, 1):
                rr[l] = work.tile([128, 1], F32, name=f"tr{l}", tag="r",
                                  bufs=4)
                nc.vector.reciprocal(rr[l][:],
                                     ps[l][:, 2 * TCH + 340:2 * TCH + 341])

            def norm_combine(cc, lo, hi):
                # hi/lo are channel bounds within [cc*CCH, (cc+1)*CCH)
                w = hi - lo
                pslc = slice(cc * TCH + (lo - cc * CCH),
                             cc * TCH + (lo - cc * CCH) + w)
                # both STTs on DVE (GPSIMD has no PSUM port); add on Pool
                nc.vector.scalar_tensor_tensor(
                    out=accs[0][:, lo:hi], in0=ps[0][:, pslc],
                    scalar=rr[0][:], in1=wh_sb[:, 0, lo:hi],
                    op0=MUL, op1=MUL)
                nc.vector.scalar_tensor_tensor(
                    out=accs[1][:, lo:hi], in0=ps[1][:, pslc],
                    scalar=rr[1][:], in1=wh_sb[:, 1, lo:hi],
                    op0=MUL, op1=MUL)
                nc.gpsimd.tensor_add(out=ybf[:, lo:hi], in0=accs[0][:, lo:hi],
                                     in1=accs[1][:, lo:hi])
                nc.sync.dma_start(y_d[b, qb * 128:(qb + 1) * 128, lo:hi],
                                  ybf[:, lo:hi])

            norm_combine(2, 2 * CCH, C)     # channels 684..1023 (skip ones)
            for cc in (0, 1):
                for l in (0, 1):
                    mm_chunk(l, cc)
                norm_combine(cc, cc * CCH, (cc + 1) * CCH)

        # ---- emission schedule ----
        def mark(name):
            MARKERS.append((name, int(nc.next_id())))

        mark("proj03")
        proj_tc(0, xt=xt0)
        for tci in range(1, 4):
            proj_tc(tci)
        # b0 QK tiles that fit ACT capacity during the proj phase
        for j in range(4):
            queue_qk(0, 0, j)
            queue_qk(0, 1, j)
        queue_qk(0, 0, 4)
        queue_qk(0, 0, 5)
        mark("proj47")
        for tci in range(4, NTC):
            proj_tc(tci)
        nc.sync.dma_start(wh_sb[:], wh_d.rearrange("l p c -> p l c"))
        load_xaug(0, 0)
        xtp.release()
        pjps.release()
        avps = tc.alloc_tile_pool(name="avps", bufs=2, space="PSUM")
        freshp = tc.alloc_tile_pool(name="freshp", bufs=1)
        for l in range(HPC):
            fresh[l] = freshp.tile([128, FRESH_COLS], BF, name=f"fresh{l}")

        # rest of b0's QK, interleaved by need time (l0 row runs first)
        queue_qk(0, 1, 4)
        queue_qk(0, 1, 5)
        for j in range(6, NT):
            queue_qk(0, 0, j)
            queue_qk(0, 1, j)

        mark("fused_b0")
        for qb in range(NT):
            if qb == 12:
                # prefetch b1's first QK tiles into the fresh planes while
                # b0's AV phase is dense (no deps on the shared planes)
                for j in range(FRESH_J):
                    queue_qk(1, 0, j)
                    queue_qk(1, 1, j)
            if qb == NT - 1:
                # b1's first shared-plane head-0 QK tiles: their chunks drip
                # inside b0's last head-0 AV row. A chunk of tile j popped at
                # st-slot s only overwrites blocks whose b0 readers were
                # emitted at st<=j < s, so per-slice WAR deps stay correct.
                # Head-1 tiles must NOT be enqueued yet: this row's head-1
                # reader is emitted after the head-0 row.
                queue_qk(1, 0, FRESH_J)
                queue_qk(1, 0, FRESH_J + 1)
            if qb % 4 == 2:
                g = qb // 4 + 1
                if g < 4:
                    load_xaug(0, g)
                load_xaug(1, g - 1)
            if qb == NT - 2:
                load_xaug(1, 3)
            acc = work.tile([128, CAUG], F32, name=f"acc0{qb}", tag="acc",
                            bufs=2)
            tmp2 = work.tile([128, CAUG], F32, name=f"tmp20{qb}",
                             tag="tmp2", bufs=2)
            need(0, 0, qb)
            av_row(0, qb, 0, acc)
            drip(1)
            if qb == NT - 1:
                # now that the head-0 row is emitted, b1's head-1 tiles may
                # drip inside the head-1 row (same slot-after-reader rule);
                # the rest interleaves by need time in b1's loop
                queue_qk(1, 1, FRESH_J)
                queue_qk(1, 1, FRESH_J + 1)
                for j in range(FRESH_J + 2, NT):
                    queue_qk(1, 0, j)
                    queue_qk(1, 1, j)
            need(0, 1, qb)
            av_row(0, qb, 1, tmp2)
            combine_store(0, qb, acc, tmp2)
            drip(1)

        mark("fused_b1")
        for qb in range(NT - 1):
            acc = work.tile([128, CAUG], F32, name=f"acc1{qb}", tag="acc",
                            bufs=2)
            tmp2 = work.tile([128, CAUG], F32, name=f"tmp21{qb}",
                             tag="tmp2", bufs=2)
            need(1, 0, qb)
            av_row(1, qb, 0, acc)
            drip(1)
            need(1, 1, qb)
            av_row(1, qb, 1, tmp2)
            combine_store(1, qb, acc, tmp2)
            drip(1)
        tail_row(1, NT - 1)
        mark("end")
        avps.release()
        freshp.release()


_CACHE = {}
MARKERS = []


def _build():
    if "nc" in _CACHE:
        return _CACHE["nc"]
    nc = bacc.Bacc("TRN2", target_bir_lowering=False, debug=False,
                   enable_asserts=False, num_devices=NCORES)
    xTt_d = nc.dram_tensor("xTt", [NTC, 128, NCT, TCH], BF,
                           kind="ExternalInput").ap()
    xaug_d = nc.dram_tensor("xaug", [B, 128, NT, CAUG], BF,
                            kind="ExternalInput").ap()
    wqk_d = nc.dram_tensor("wqk", [2, 128, NCT, 128], BF,
                           kind="ExternalInput").ap()
    bqk_d = nc.dram_tensor("bqk", [2, 128], F32, kind="ExternalInput").ap()
    wh_d = nc.dram_tensor("wh", [HPC, 128, CAUG], F32, kind="ExternalInput").ap()
    mask_d = nc.dram_tensor("mask", [128, 128], BF, kind="ExternalInput").ap()
    y_d = nc.dram_tensor("y", [B, T, C], BF, kind="ExternalOutput").ap()
    with tile.TileContext(nc, trace_sim=False) as tc:
        _emit(nc, xTt_d, xaug_d, wqk_d, bqk_d, wh_d, mask_d, y_d, tc)
    nc.compile()
    _CACHE["nc"] = nc
    return nc


def _prep_inputs(x, W_attn, b_attn, head_weights):
    x = np.asarray(x, dtype=np.float32)
    W_attn = np.asarray(W_attn, dtype=np.float32)
    b_attn = np.asarray(b_attn, dtype=np.float32)
    head_weights = np.asarray(head_weights, dtype=np.float32)

    xf = x.reshape(B * T, C)
    # xTt[tc, p, ct, u] = x[tc*512+u, ct*128+p]
    xTt = np.ascontiguousarray(
        xf.reshape(NTC, TCH, NCT, 128).transpose(0, 3, 2, 1)).astype(BF16)
    xaug = np.zeros((B, T, CAUG), dtype=np.float32)
    xaug[:, :, :C] = x
    xaug[:, :, C] = 1.0
    xaug = np.ascontiguousarray(
        xaug.reshape(B, NT, 128, CAUG).transpose(0, 2, 1, 3)).astype(BF16)
    mask = np.triu(np.ones((128, 128), dtype=np.float32)).astype(BF16)

    in_maps = []
    for core in range(NCORES):
        h0 = HPC * core
        cols = np.concatenate(
            [np.arange(h * HS, (h + 1) * HS) for h in range(h0, h0 + HPC)])
        wq = W_attn[:, cols]          # [1024, 128]
        wk = W_attn[:, C + cols]
        # wqk[qk, p, ct, m] = W[ct*128+p, m]
        wqk = np.stack([
            np.ascontiguousarray(w.reshape(NCT, 128, 128).transpose(1, 0, 2))
            for w in (wq, wk)]).astype(BF16)
        bqk = np.stack([b_attn[cols], b_attn[C + cols]]).astype(np.float32)
        whp = np.zeros((HPC, CAUG), dtype=np.float32)
        whp[:, :C] = head_weights[h0:h0 + HPC]
        wh = np.ascontiguousarray(
            np.broadcast_to(whp[:, None, :], (HPC, 128, CAUG))
        ).astype(np.float32)
        in_maps.append({
            "xTt": xTt, "xaug": xaug, "mask": mask,
            "wqk": wqk, "bqk": bqk, "wh": wh,
        })
    return in_maps


def _run(inputs, trace=False, **kwargs):
    nc = _build()
    in_maps = _prep_inputs(**inputs)
    res = run_bass_kernel_spmd(nc, in_maps, core_ids=list(range(NCORES)),
                               trace=trace, **kwargs)
    y = np.zeros((B, T, C), dtype=np.float64)
    for core in range(NCORES):
        y += res.results[core]["y"].astype(np.float64)
    return y.astype(np.float32), res


def kernel(x, W_attn, b_attn, head_weights):
    y, _ = _run(dict(x=x, W_attn=W_attn, b_attn=b_attn,
                     head_weights=head_weights))
    return y
